# revision 1
# baseline (speedup 1.0000x reference)
"""Mean-IoU kernel for Trainium2, SPMD over 8 NeuronCores.

Strategy (data-parallel over batch N=16, 2 images per core):
  - Host pre-transposes inputs to (N, H, W, C) so the on-chip layout is
    pixels-on-partitions with classes innermost: x tile (128, F, 19) f32,
    fully contiguous for DMA (19456B runs), reduce and one-hot.
  - Per tile: m = reduce_max over the contiguous class axis (DVE), then
    one-hot zb = (x == m) as bf16, contiguous writes (DVE tensor_tensor
    is_equal with an innermost-broadcast max operand).
  - TensorE bf16 matmuls Zp^T @ Zt accumulate a block confusion matrix
    (JB=6 pixel-columns -> 114x114 PSUM) per image.
  - Host: sum jb-diagonal 19x19 blocks -> confusion M; pred = M.sum(1),
    targ = M.sum(0), inter = diag(M); IoU + means (tiny, exact).
"""
import os
import sys

for _p in ('/opt/trn_rl_repo', '/root/.axon_site/_ro/trn_rl_repo'):
    if os.path.isdir(_p) and _p not in sys.path:
        sys.path.insert(0, _p)

import numpy as np

# problem constants (hardcoded per contest rules)
N_FULL = 16
C = 19
H = 512
W = 512
HW = H * W
EPS = 1e-06

N_CORES = 8
N_LOC = N_FULL // N_CORES      # 2 images per core
P = 128                        # SBUF partitions = pixel groups
Q = HW // P                    # 2048 pixels per partition
F = 256                        # pixels per partition per tile
N_TILES = Q // F               # 8 tiles per image
JB = 6                         # pixel-columns per confusion matmul
NCOLS = JB * C                 # 114
# of the 32 tensor-tiles per core, how many route the one-hot through
# GPSIMD subtract (contiguous bf16 d) + DVE tensor_scalar is_equal (4x)
K_RD = int(os.environ.get("KERNEL_K_RD", "0"))

_CACHE = {}


def _build_nc():
    from concourse import bacc, tile, mybir

    nc = bacc.Bacc("TRN2", target_bir_lowering=False, debug=False,
                   num_devices=N_CORES)
    # host-transposed layout: (n, h*w, c)
    preds = nc.dram_tensor("preds", (N_LOC, HW, C), mybir.dt.float32,
                           kind="ExternalInput")
    targs = nc.dram_tensor("targets", (N_LOC, HW, C), mybir.dt.float32,
                           kind="ExternalInput")
    conf_out = nc.dram_tensor("conf", (N_LOC, NCOLS, NCOLS), mybir.dt.float32,
                              kind="ExternalOutput")

    pv = preds.ap().rearrange("n (p j) c -> n p j c", p=P)
    tv = targs.ap().rearrange("n (p j) c -> n p j c", p=P)

    n_tt = 2 * N_TILES * N_LOC
    rd_route = []
    acc = 0
    for i in range(n_tt):
        nacc = (i + 1) * K_RD // n_tt
        rd_route.append(nacc > acc)
        acc = nacc

    with tile.TileContext(nc) as tc:
        with (
            tc.tile_pool(name="sbuf", bufs=2) as pool,
            tc.tile_pool(name="psum", bufs=2, space="PSUM") as psum_pool,
        ):
            tt_idx = 0
            for n in range(N_LOC):
                conf = psum_pool.tile([NCOLS, NCOLS], mybir.dt.float32)
                for t in range(N_TILES):
                    zbs = {}
                    for name, dview in (("p", pv), ("t", tv)):
                        x = pool.tile([P, F, C], mybir.dt.float32,
                                      tag=f"x{name}")
                        nc.sync.dma_start(x[:], dview[n, :,
                                                      t * F:(t + 1) * F, :])
                        m = pool.tile([P, F], mybir.dt.float32, tag=f"m{name}")
                        nc.vector.reduce_max(m[:], x[:],
                                             axis=mybir.AxisListType.X)
                        zb = pool.tile([P, F, C], mybir.dt.bfloat16,
                                       tag=f"zb{name}")
                        mb = m[:, :, None].broadcast_to((P, F, C))
                        if rd_route[tt_idx]:
                            dbf = pool.tile([P, F, C], mybir.dt.bfloat16,
                                            tag="dbf")
                            nc.gpsimd.tensor_tensor(
                                dbf[:], x[:], mb,
                                op=mybir.AluOpType.subtract)
                            nc.vector.tensor_scalar(
                                zb[:].rearrange("p j c -> p (j c)"),
                                dbf[:].rearrange("p j c -> p (j c)"),
                                0.0, None, op0=mybir.AluOpType.is_equal)
                        else:
                            nc.vector.tensor_tensor(
                                zb[:], x[:], mb,
                                op=mybir.AluOpType.is_equal)
                        zbs[name] = zb
                        tt_idx += 1
                    zpf = zbs["p"][:].rearrange("p j c -> p (j c)")
                    ztf = zbs["t"][:].rearrange("p j c -> p (j c)")
                    nmm = (F + JB - 1) // JB            # 43 (42 full + 1 of 4)
                    for b in range(nmm):
                        cols = min(JB, F - b * JB) * C
                        first = (t == 0 and b == 0)
                        last = (t == N_TILES - 1 and b == nmm - 1)
                        nc.tensor.matmul(conf[0:cols, 0:cols],
                                         zpf[:, b * NCOLS:b * NCOLS + cols],
                                         ztf[:, b * NCOLS:b * NCOLS + cols],
                                         start=first, stop=last)
                sb = pool.tile([NCOLS, NCOLS], mybir.dt.float32, tag="confsb")
                nc.scalar.copy(sb[:], conf[:])
                nc.sync.dma_start(conf_out.ap()[n], sb[:])

    nc.compile()
    return nc


def _get_nc():
    if "nc" not in _CACHE:
        _CACHE["nc"] = _build_nc()
    return _CACHE["nc"]


def run_on_hw(preds, targets, trace=False):
    """Run the SPMD kernel; returns (conf (16,NCOLS,NCOLS) np.f32, results)."""
    from concourse.bass_utils import run_bass_kernel_spmd

    nc = _get_nc()
    # (N, C, H, W) -> (N, H*W, C) contiguous, so every device access is
    # contiguous (DMA runs, class-axis reduce, one-hot writes)
    preds = np.ascontiguousarray(
        np.asarray(preds, dtype=np.float32).reshape(N_FULL, C, HW)
        .transpose(0, 2, 1))
    targets = np.ascontiguousarray(
        np.asarray(targets, dtype=np.float32).reshape(N_FULL, C, HW)
        .transpose(0, 2, 1))
    in_maps = [
        {"preds": preds[i * N_LOC:(i + 1) * N_LOC],
         "targets": targets[i * N_LOC:(i + 1) * N_LOC]}
        for i in range(N_CORES)
    ]
    res = run_bass_kernel_spmd(nc, in_maps, core_ids=list(range(N_CORES)),
                               trace=trace)
    conf = np.concatenate([res.results[i]["conf"] for i in range(N_CORES)],
                          axis=0)
    return conf, res


def postprocess(conf, class_weights):
    """conf: (16, NCOLS, NCOLS) block confusion -> scalar mean IoU."""
    conf = conf.astype(np.float64)
    M = np.zeros((N_FULL, C, C))
    for k in range(JB):
        M += conf[:, k * C:(k + 1) * C, k * C:(k + 1) * C]
    inter = np.diagonal(M, axis1=1, axis2=2)          # (N, C)
    pred_cnt = M.sum(axis=2)                          # (N, C)
    targ_cnt = M.sum(axis=1)                          # (N, C)
    union = pred_cnt + targ_cnt - inter
    iou = (inter + EPS) / (union + EPS)
    weighted = iou * np.asarray(class_weights, dtype=np.float64)[None, :]
    return np.float32(weighted.mean())


def kernel(preds, targets, class_weights):
    conf, _ = run_on_hw(preds, targets, trace=False)
    return postprocess(conf, class_weights)

